# revision 19
# baseline (speedup 1.0000x reference)
"""Trainium2 Bass kernel for nn_HandIntersectionLoss — single-core edition.

Why single core: this environment reaches the NeuronCores through an
axon tunnel whose blocking round trip costs ~70-90 ms regardless of
payload, and every extra device adds sequential per-device RPCs.  The
whole computation (64 batches x 2 directions x 251 points x 500 faces)
is only ~2-3 ms of device time, so wall time is pure dispatch latency.
One core + minimal input bytes + a single cached jit dispatch
minimizes the number of round trips.

Device-side structure (per call):
  inputs: ptsK [128,4,256] f32 (hand points as [x,y,z,|p|^2] planes,
          bd = dir*64 + batch, padded with far-away points), faces
          [2,3*512] (corner-major vertex ids as f32), qiota/irow
          (tiny iota constants).
  prep:   PE-transpose ptsK -> vertex columns; build one-hot gather
          matrices from faces via iota compare; gather triangle
          corners with matmuls; DVE algebra expands per-face constants
          (corners, midpoints, dots, normal n=(B-A)x(C-A), det) into
          ALG [128,5,7,512]; write to an internal-DRAM frhs buffer.
          Also derive the moving-point rows S=[-2p,1,|p|^2] and
          min-dist columns M=[v,|v|^2,1] in SBUF.
  main:   identical math to the proven 8-core kernel: per (bd, chunk)
          block K=5 matmuls produce |a|^2,|b|^2,|c|^2,ab,bc,ca,det for
          128 points x 500 faces, DVE/ACT assemble the Van Oosterom /
          Strackee denominator and the range-reduced atan2
          (two ACT-table passes, supergroups of 8 blocks; pass B is
          batched over the supergroup).  Winding sum > pi/2 gates the
          nearest-vertex depth; per-block partition-reduce matmuls
          produce loss [128,2].
Host does only index gathers (O(B*V)) and the final unscramble.
"""
import os
import sys
import numpy as np

sys.path.insert(0, '/opt/trn_rl_repo')

B, V_FULL, V_HAND, V_LOOP, N_FACES = 64, 6890, 250, 20, 500
P = V_HAND + 1          # 251 points/verts per hand (incl. lid)
PPAD = 256
F = N_FACES
FP = 512
NBD = 2 * B             # 128 (bd = d*64 + b)
NBLK = 2 * NBD          # 256 point-chunk blocks
SUPER = 8               # blocks per two-pass super-group
NSG = NBLK // SUPER
HALF_PI = float(np.pi / 2)
PADV = 1.0e3            # pad points far away

# single f32 input blob layout (offsets in elements); fp16 inputs were
# tried and rejected: coordinate quantization flips inside/outside for
# points near the other hand's surface (rel err 5.9e-2 > 2e-2 budget)
L_PTS = NBD * 4 * PPAD          # 131072
L_FACES = 3 * FP                # 1536 per hand
OFF_FACES = L_PTS
OFF_QI = OFF_FACES + 2 * L_FACES
OFF_IR = OFF_QI + 256
L_BLOB = OFF_IR + 128           # 134528

_cache = {}
last_exec_time_ns = None


# --------------------------------------------------------------------------
# host prep: index gathers only
# --------------------------------------------------------------------------

def _host_prep(inputs):
    verts = np.asarray(inputs['verts_batch'], dtype=np.float32)
    hi = (np.asarray(inputs['hand_verts_inds_left'], dtype=np.int64),
          np.asarray(inputs['hand_verts_inds_right'], dtype=np.int64))
    li = (np.asarray(inputs['hand_loop_verts_inds_left'], dtype=np.int64),
          np.asarray(inputs['hand_loop_verts_inds_right'], dtype=np.int64))
    fa = (np.asarray(inputs['hand_faces_left'], dtype=np.int64),
          np.asarray(inputs['hand_faces_right'], dtype=np.int64))

    ptsK = np.full((NBD, 4, PPAD), PADV, np.float32)
    ptsK[:, 3, :] = 3.0 * PADV * PADV
    for d in range(2):
        h = verts[:, hi[d]]                                     # [B,250,3]
        lid = verts[:, li[d]].mean(axis=1, keepdims=True)       # [B,1,3]
        p = np.concatenate([h, lid], axis=1)                    # [B,251,3]
        sl = slice(d * B, d * B + B)
        ptsK[sl, 0:3, :P] = p.transpose(0, 2, 1)
        ptsK[sl, 3, :P] = (p * p).sum(-1)

    blob = _cache.setdefault("blob", np.empty(L_BLOB, np.float32))
    blob[:L_PTS] = ptsK.reshape(-1)
    fb = np.full((2, 3, FP), -1.0, np.float32)
    for h_ in range(2):
        fb[h_, :, :F] = fa[h_].T
    blob[OFF_FACES:OFF_QI] = fb.reshape(-1)
    if not _cache.get("blob_const"):
        qiota = np.stack([np.arange(128, dtype=np.float32),
                          np.arange(128, 256, dtype=np.float32)], axis=1)
        blob[OFF_QI:OFF_IR] = qiota.reshape(-1)
        blob[OFF_IR:] = np.arange(128, dtype=np.float32)
        _cache["blob_const"] = True
    return {"blob": blob}


# --------------------------------------------------------------------------
# device kernel
# --------------------------------------------------------------------------

def _kernel_body(tc, blob_d, loss_d):
    import concourse.mybir as mybir
    nc = tc.nc
    fp32 = mybir.dt.float32
    AF = mybir.ActivationFunctionType
    OP = mybir.AluOpType
    AX = mybir.AxisListType.X
    TT = nc.vector.tensor_tensor
    TS = nc.vector.tensor_scalar
    STT = nc.vector.scalar_tensor_tensor

    with (
        tc.tile_pool(name="const", bufs=1) as cpool,
        tc.tile_pool(name="dram", bufs=1, space="DRAM") as dpool,
    ):
        frhs_dram = dpool.tile([NBD, 5, 7, FP], fp32)

        ptsK = cpool.tile([128, 4, PPAD], fp32)
        S = cpool.tile([128, 5, PPAD], fp32)     # lhsT rows per bd (bd-major)
        M = cpool.tile([128, 5, PPAD], fp32)     # min-dist cols per bd
        qiota = cpool.tile([128, 2], fp32)
        ones = cpool.tile([128, 1], fp32)
        sacc = cpool.tile([128, NBLK], fp32)
        minda = cpool.tile([128, NBLK], fp32)

        nc.sync.dma_start(ptsK[:], blob_d[0:L_PTS])
        nc.sync.dma_start(qiota[:], blob_d[OFF_QI:OFF_QI + 256])
        nc.vector.memset(ones[:], 1.0)

        TS(S[:, 0:3, :], ptsK[:, 0:3, :], -2.0, None, OP.mult)
        nc.vector.memset(S[:, 3, :], 1.0)
        nc.scalar.copy(S[:, 4, :], ptsK[:, 3, :])
        nc.scalar.copy(M[:, 0:4, :], ptsK[:, 0:4, :])
        nc.vector.memset(M[:, 4, :], 1.0)

        # ---------------- prep: per-face constants -> frhs_dram ------------
        with (
            tc.tile_pool(name="prep", bufs=1) as pr,
            tc.tile_pool(name="prep2", bufs=2) as pr2,
            tc.tile_pool(name="prep_ps", bufs=1, space="PSUM") as pp,
        ):
            faces0 = pr.tile([1, 3 * FP], fp32)
            faces1 = pr.tile([1, 3 * FP], fp32)
            irow = pr.tile([1, 128], fp32)
            onesr = pr.tile([1, 128], fp32)
            nc.sync.dma_start(faces0[:], blob_d[OFF_FACES:OFF_FACES + L_FACES])
            nc.sync.dma_start(
                faces1[:], blob_d[OFF_FACES + L_FACES:OFF_FACES + 2 * L_FACES])
            nc.sync.dma_start(irow[:], blob_d[OFF_IR:OFF_IR + 128])
            nc.vector.memset(onesr[:], 1.0)

            # identity (for PE transpose) from iota compare
            psumI = pp.tile([128, 128], fp32, tag="pI")
            nc.tensor.matmul(psumI[:], onesr[:], irow[:])
            ident = pr.tile([128, 128], fp32)
            TS(ident[:], psumI[:], qiota[:, 0:1], None, OP.is_equal)

            # gvo[qhat, t, bd]: vertex values transposed; t = 2*coord + half
            gvo = pr.tile([128, 6, 128], fp32)
            for t in range(6):
                pt = pp.tile([128, 128], fp32, tag="pT")
                nc.tensor.transpose(
                    pt[:], ptsK[:, t // 2, (t % 2) * 128:(t % 2) * 128 + 128],
                    ident[:])
                nc.scalar.copy(gvo[:, t, :], pt[:])

            ALG = pr.tile([128, 5, 7, FP], fp32)

            for h in range(2):          # h = hand whose faces we gather
                d = 1 - h               # ALG dir-block receiving them
                rows = slice(d * B, d * B + B)
                cols = slice(h * B, h * B + B)
                fsb = faces0 if h == 0 else faces1
                psumF = pp.tile([128, 3, FP], fp32, tag="pF")
                for nsp in range(3):
                    nc.tensor.matmul(psumF[:, nsp, :], onesr[:],
                                     fsb[:, nsp * FP:(nsp + 1) * FP])
                G0 = pr.tile([128, 3, FP], fp32, tag="G0")
                G1 = pr.tile([128, 3, FP], fp32, tag="G1")
                TS(G0[:], psumF[:], qiota[:, 0:1], None, OP.is_equal)
                TS(G1[:], psumF[:], qiota[:, 1:2], None, OP.is_equal)
                G = (G0, G1)
                for k in range(3):
                    psumT = pp.tile([B, 3, FP], fp32, tag="pTRI")
                    for nsp in range(3):
                        for c2 in range(2):
                            nc.tensor.matmul(psumT[:, nsp, :],
                                             gvo[:, 2 * k + c2, cols],
                                             G[c2][:, nsp, :],
                                             start=(c2 == 0), stop=(c2 == 1))
                    for g in range(3):
                        nc.scalar.copy(ALG[rows, k, g, :], psumT[:, g, :])

            def Ap(k): return ALG[:, k, 0, :]
            def Bp(k): return ALG[:, k, 1, :]
            def Cp(k): return ALG[:, k, 2, :]

            # midpoint groups (A+B)/2, (B+C)/2, (C+A)/2
            for g, (X, Y) in ((3, (Ap, Bp)), (4, (Bp, Cp)), (5, (Cp, Ap))):
                for k in range(3):
                    t1 = pr2.tile([128, FP], fp32, tag="mid")
                    TT(t1[:], X(k), Y(k), OP.add)
                    TS(ALG[:, k, g, :], t1[:], 0.5, None, OP.mult)

            # row 3: |A|^2,|B|^2,|C|^2, A.B, B.C, C.A
            for g, (X, Y) in enumerate(((Ap, Ap), (Bp, Bp), (Cp, Cp),
                                        (Ap, Bp), (Bp, Cp), (Cp, Ap))):
                t1 = pr2.tile([128, FP], fp32, tag="d1")
                t2 = pr2.tile([128, FP], fp32, tag="d2")
                t3 = pr2.tile([128, FP], fp32, tag="d3")
                t4 = pr2.tile([128, FP], fp32, tag="d4")
                TT(t1[:], X(0), Y(0), OP.mult)
                TT(t2[:], X(1), Y(1), OP.mult)
                TT(t3[:], t1[:], t2[:], OP.add)
                TT(t4[:], X(2), Y(2), OP.mult)
                TT(ALG[:, 3, g, :], t3[:], t4[:], OP.add)

            # group 6: n = (B-A)x(C-A) = AxB+BxC+CxA, d0 = A.n
            E1 = pr.tile([128, 3, FP], fp32)
            E2 = pr.tile([128, 3, FP], fp32)
            N3 = pr.tile([128, 3, FP], fp32)
            for k in range(3):
                TT(E1[:, k, :], Bp(k), Ap(k), OP.subtract)
                TT(E2[:, k, :], Cp(k), Ap(k), OP.subtract)
            for k, (i1, i2) in ((0, (1, 2)), (1, (2, 0)), (2, (0, 1))):
                c1 = pr2.tile([128, FP], fp32, tag="cx1")
                c2_ = pr2.tile([128, FP], fp32, tag="cx2")
                TT(c1[:], E1[:, i1, :], E2[:, i2, :], OP.mult)
                TT(c2_[:], E1[:, i2, :], E2[:, i1, :], OP.mult)
                TT(N3[:, k, :], c1[:], c2_[:], OP.subtract)
            d1 = pr2.tile([128, FP], fp32, tag="d1")
            d2 = pr2.tile([128, FP], fp32, tag="d2")
            d3 = pr2.tile([128, FP], fp32, tag="d3")
            d4 = pr2.tile([128, FP], fp32, tag="d4")
            TT(d1[:], Ap(0), N3[:, 0, :], OP.mult)
            TT(d2[:], Ap(1), N3[:, 1, :], OP.mult)
            TT(d3[:], d1[:], d2[:], OP.add)
            TT(d4[:], Ap(2), N3[:, 2, :], OP.mult)
            TT(ALG[:, 3, 6, :], d3[:], d4[:], OP.add)
            for k in range(3):
                TS(ALG[:, k, 6, :], N3[:, k, :], 0.5, None, OP.mult)

            nc.vector.memset(ALG[:, 4, 0:6, :], 1.0)
            nc.vector.memset(ALG[:, 4, 6, :], 0.0)

            nc.sync.dma_start(frhs_dram[:], ALG[:])

        # ---------------- main loop (proven 8-core math, 256 blocks) -------
        with (
            tc.tile_pool(name="store", bufs=1) as spool,
            tc.tile_pool(name="stage", bufs=2) as stpool,
            tc.tile_pool(name="iface", bufs=2) as ipool,
            tc.tile_pool(name="dve", bufs=1) as vpool,
        ):
            denoms = spool.tile([128, SUPER, FP], fp32)
            tts = spool.tile([128, SUPER, FP], fp32)
            bx = spool.tile([128, SUPER, FP], fp32)
            by = spool.tile([128, SUPER, FP], fp32)
            bz = spool.tile([128, SUPER, FP], fp32)

            def pass_a(ppool, lstage, bd0, i, j):
                bd, ch = divmod(i, 2)
                if ch == 0:
                    fstage = stpool.tile([5, 7, FP], fp32, tag="fstage")
                    mstage = stpool.tile([5, PPAD], fp32, tag="mstage")
                    nc.sync.dma_start(fstage[:], frhs_dram[bd])
                    other = (bd + B) % NBD
                    nc.sync.dma_start(mstage[:], M[other:other + 1, :, :])
                    pass_a.stage = (fstage, mstage)
                fstage, mstage = pass_a.stage
                lhs = lstage[:, bd - bd0, ch * 128:(ch + 1) * 128]

                wind = ppool.tile([128, 7, FP], fp32, tag="wind")
                md = ppool.tile([128, PPAD], fp32, tag="md")
                for g in range(7):
                    nc.tensor.matmul(wind[:, g, :], lhs, fstage[:, g, :])
                nc.tensor.matmul(md[:, :P], lhs, mstage[:, :P])

                mind = vpool.tile([128, 1], fp32, tag="mind")
                nc.vector.tensor_reduce(mind[:], md[:, :P], AX, OP.min)
                TS(minda[:, i:i + 1], mind[:], 0.0, None, OP.max)

                rl = ipool.tile([128, 3, FP], fp32, tag="rl")
                rl2 = ipool.tile([128, 3, FP], fp32, tag="rl2")
                nc.scalar.activation(rl[:], wind[:, 0:3, :], AF.Relu)
                nc.scalar.activation(rl2[:], rl[:], AF.Sqrt)
                dets = ipool.tile([128, FP], fp32, tag="dets")
                nc.scalar.copy(dets[:], wind[:, 6, :])

                r4 = vpool.tile([128, FP], fp32, tag="r4")
                s5 = vpool.tile([128, FP], fp32, tag="s5")
                u = vpool.tile([128, FP], fp32, tag="u")
                v = vpool.tile([128, FP], fp32, tag="v")
                w = vpool.tile([128, FP], fp32, tag="w")
                t6 = vpool.tile([128, FP], fp32, tag="t6")
                TT(r4[:], wind[:, 4, :], rl2[:, 0, :], OP.mult)
                TT(s5[:], wind[:, 5, :], rl2[:, 1, :], OP.mult)
                TT(u[:], rl2[:, 0, :], rl2[:, 1, :], OP.mult)
                TT(v[:], u[:], wind[:, 3, :], OP.add)
                TT(w[:], v[:], rl2[:, 2, :], OP.mult)
                TT(t6[:], r4[:], s5[:], OP.add)
                den = denoms[:, j, :]
                TT(den, w[:], t6[:], OP.add)

                xx = ipool.tile([128, FP], fp32, tag="xx")
                yy = ipool.tile([128, FP], fp32, tag="yy")
                ss = vpool.tile([128, FP], fp32, tag="ss", bufs=2)
                rho = ipool.tile([128, FP], fp32, tag="rho")
                axd = ipool.tile([128, FP], fp32, tag="axd")
                dd = vpool.tile([128, FP], fp32, tag="dd")
                rd = vpool.tile([128, FP], fp32, tag="rd")
                nc.scalar.activation(xx[:], den, AF.Square)
                nc.scalar.activation(yy[:], dets[:], AF.Square)
                STT(ss[:], xx[:], 1e-20, yy[:], OP.add, OP.add)
                nc.scalar.activation(rho[:], ss[:], AF.Sqrt)
                nc.scalar.activation(axd[:], den, AF.Abs)
                TT(dd[:], rho[:], axd[:], OP.add)
                nc.vector.reciprocal_approx_fast(rd[:], dd[:])
                TT(tts[:, j, :], dets[:], rd[:], OP.mult)

            def pass_b(s):
                # atan2/2 = atn + [den<0]*(sign(det)*pi/2 - 2*atn), batched
                nc.scalar.activation(bx[:], tts[:], AF.Arctan)
                nc.scalar.activation(by[:], tts[:], AF.Sign)
                nc.scalar.mul(bz[:], by[:], HALF_PI)
                STT(by[:], bx[:], -2.0, bz[:], OP.mult, OP.add)
                STT(bz[:], denoms[:], 0.0, by[:], OP.is_lt, OP.mult)
                TT(by[:], bx[:], bz[:], OP.add)
                nc.vector.tensor_reduce(sacc[:, s * SUPER:(s + 1) * SUPER],
                                        by[:], AX, OP.add)

            with tc.tile_pool(name="psum", bufs=1, space="PSUM") as ppool:
                for s in range(NSG):
                    bd0 = s * (SUPER // 2)
                    lstage = stpool.tile([5, SUPER // 2, PPAD], fp32,
                                         tag="lstage")
                    for o in range(SUPER // 2):
                        nc.sync.dma_start(lstage[:, o, :],
                                          S[bd0 + o:bd0 + o + 1, :, :])
                    for j in range(SUPER):
                        pass_a(ppool, lstage, bd0, s * SUPER + j, j)
                    tc.no_sync_barrier()
                    pass_b(s)
                    tc.no_sync_barrier()

            # ---------------- final: depth * inside, partition-reduce ------
            inside = cpool.tile([128, NBLK], fp32)
            depth = cpool.tile([128, NBLK], fp32)
            contrib = cpool.tile([128, NBLK], fp32)
            beps = cpool.tile([128, 1], fp32)
            nc.vector.memset(beps[:], 1e-12)
            TS(inside[:], sacc[:], HALF_PI, None, OP.is_gt)
            nc.scalar.activation(depth[:], minda[:], AF.Sqrt, bias=beps[:])
            TT(contrib[:], depth[:], inside[:], OP.mult)

            with tc.tile_pool(name="psum2", bufs=1, space="PSUM") as p2:
                lp0 = p2.tile([128, 1], fp32, tag="lp0")
                lp1 = p2.tile([128, 1], fp32, tag="lp1")
                nc.tensor.matmul(lp0[:], contrib[:, 0:128], ones[:])
                nc.tensor.matmul(lp1[:], contrib[:, 128:256], ones[:])
                loss_sb = cpool.tile([128, 2], fp32)
                nc.scalar.copy(loss_sb[:, 0:1], lp0[:])
                nc.scalar.copy(loss_sb[:, 1:2], lp1[:])
                nc.sync.dma_start(loss_d[:], loss_sb[:])


def _build():
    if "nc" in _cache:
        return _cache["nc"]
    import concourse.bacc as bacc
    import concourse.mybir as mybir
    import concourse.tile as tile

    nc = bacc.Bacc("TRN2", target_bir_lowering=False, debug=False,
                   num_devices=1)
    fp32 = mybir.dt.float32
    blob_d = nc.dram_tensor("blob", (L_BLOB,), fp32,
                            kind="ExternalInput").ap()
    loss_d = nc.dram_tensor("loss", (128, 2), fp32,
                            kind="ExternalOutput").ap()

    with tile.TileContext(nc) as tc:
        _kernel_body(tc, blob_d, loss_d)
    nc.compile()
    _cache["nc"] = nc
    return nc


# --------------------------------------------------------------------------
# cached single-device dispatch (PJRT via axon)
# --------------------------------------------------------------------------

def _get_runner(nc):
    if "run" in _cache:
        return _cache["run"]
    import jax
    from concourse import bass2jax, mybir
    from concourse.bass2jax import _bass_exec_p, partition_id_tensor

    bass2jax.install_neuronx_cc_hook()
    partition_name = (nc.partition_id_tensor.name
                      if nc.partition_id_tensor else None)
    in_names, out_names, out_avals = [], [], []
    for alloc in nc.m.functions[0].allocations:
        if not isinstance(alloc, mybir.MemoryLocationSet):
            continue
        name = alloc.memorylocations[0].name
        if alloc.kind == "ExternalInput":
            if name != partition_name:
                in_names.append(name)
        elif alloc.kind == "ExternalOutput":
            out_names.append(name)
            out_avals.append(jax.core.ShapedArray(
                tuple(alloc.tensor_shape), mybir.dt.np(alloc.dtype)))
    n_params = len(in_names)
    in_names_full = list(in_names) + list(out_names)
    if partition_name is not None:
        in_names_full.append(partition_name)
    donate = tuple(range(n_params, n_params + len(out_names)))
    dbg_zero = {}
    if nc.dbg_addr is not None:
        assert not nc.dbg_callbacks
        dbg_zero = {nc.dbg_addr.name: np.zeros((1, 2), np.uint32)}

    def _body(*args):
        operands = list(args)
        if partition_name is not None:
            operands.append(partition_id_tensor())
        return tuple(_bass_exec_p.bind(
            *operands, out_avals=tuple(out_avals),
            in_names=tuple(in_names_full), out_names=tuple(out_names),
            lowering_input_output_aliases=(),
            sim_require_finite=True, sim_require_nnan=True, nc=nc))

    fn = jax.jit(_body, donate_argnums=donate, keep_unused=True)
    out_shapes = [(tuple(a.shape), a.dtype) for a in out_avals]

    def run(in_map):
        full = {**in_map, **dbg_zero}
        args = [np.ascontiguousarray(full[n]) for n in in_names]
        zeros = [np.zeros(sh, dt) for sh, dt in out_shapes]
        outs = fn(*args, *zeros)
        # Pipeline the D2H copy behind the execute instead of paying a
        # separate blocking round trip for it (saves ~30ms on this link).
        for o in outs:
            try:
                o.copy_to_host_async()
            except Exception:
                pass
        return {name: np.asarray(outs[k]) for k, name in enumerate(out_names)}

    _cache["run"] = run
    return run


# --------------------------------------------------------------------------
# entry point
# --------------------------------------------------------------------------

def kernel(**inputs) -> np.ndarray:
    global last_exec_time_ns
    in_map = _host_prep(inputs)
    nc = _build()
    run = _get_runner(nc)
    out = run(in_map)["loss"].astype(np.float32)        # [128, 2]
    flat = np.concatenate([out[:, 0], out[:, 1]])       # block i = bd*2+ch
    per_bd = flat.reshape(NBD, 2).sum(axis=1)
    loss = per_bd[:B] + per_bd[B:]
    last_exec_time_ns = None
    return loss.astype(np.float32)


# revision 25
# speedup vs baseline: 1.1088x; 1.1088x over previous
"""Trainium2 Bass kernel for nn_HandIntersectionLoss — single-core edition.

Why single core: this environment reaches the NeuronCores through an
axon tunnel whose blocking round trip costs ~70-90 ms regardless of
payload, and every extra device adds sequential per-device RPCs.  The
whole computation (64 batches x 2 directions x 251 points x 500 faces)
is only ~2-3 ms of device time, so wall time is pure dispatch latency.
One core + minimal input bytes + a single cached jit dispatch
minimizes the number of round trips.

Device-side structure (per call):
  inputs: ptsK [128,4,256] f32 (hand points as [x,y,z,|p|^2] planes,
          bd = dir*64 + batch, padded with far-away points), faces
          [2,3*512] (corner-major vertex ids as f32), qiota/irow
          (tiny iota constants).
  prep:   PE-transpose ptsK -> vertex columns; build one-hot gather
          matrices from faces via iota compare; gather triangle
          corners with matmuls; DVE algebra expands per-face constants
          (corners, midpoints, dots, normal n=(B-A)x(C-A), det) into
          ALG [128,5,7,512]; write to an internal-DRAM frhs buffer.
          Also derive the moving-point rows S=[-2p,1,|p|^2] and
          min-dist columns M=[v,|v|^2,1] in SBUF.
  main:   identical math to the proven 8-core kernel: per (bd, chunk)
          block K=5 matmuls produce |a|^2,|b|^2,|c|^2,ab,bc,ca,det for
          128 points x 500 faces, DVE/ACT assemble the Van Oosterom /
          Strackee denominator and the range-reduced atan2
          (two ACT-table passes, supergroups of 8 blocks; pass B is
          batched over the supergroup).  Winding sum > pi/2 gates the
          nearest-vertex depth; per-block partition-reduce matmuls
          produce loss [128,2].
Host does only index gathers (O(B*V)) and the final unscramble.
"""
import os
import sys
import numpy as np

sys.path.insert(0, '/opt/trn_rl_repo')

B, V_FULL, V_HAND, V_LOOP, N_FACES = 64, 6890, 250, 20, 500
P = V_HAND + 1          # 251 points/verts per hand (incl. lid)
PPAD = 256
F = N_FACES
FP = 512
NBD = 2 * B             # 128 (bd = d*64 + b)
NBLK = 2 * NBD          # 256 point-chunk blocks
SUPER = 8               # blocks per two-pass super-group
NSG = NBLK // SUPER
HALF_PI = float(np.pi / 2)
PADV = 1.0e3            # pad points far away

# single f32 input blob layout (offsets in elements); fp16 inputs were
# tried and rejected: coordinate quantization flips inside/outside for
# points near the other hand's surface (rel err 5.9e-2 > 2e-2 budget).
# |p|^2 is derived on device, so only the 3 coordinate planes ship.
L_PTS = NBD * 3 * PPAD          # 98304
L_FACES = 3 * FP                # 1536 per hand
OFF_FACES = L_PTS
OFF_QI = OFF_FACES + 2 * L_FACES
OFF_IR = OFF_QI + 256
L_BLOB = OFF_IR + 128           # 101760

_cache = {}
last_exec_time_ns = None


# --------------------------------------------------------------------------
# host prep: index gathers only
# --------------------------------------------------------------------------

def _host_prep(inputs):
    verts = np.asarray(inputs['verts_batch'], dtype=np.float32)
    hi = (np.asarray(inputs['hand_verts_inds_left'], dtype=np.int64),
          np.asarray(inputs['hand_verts_inds_right'], dtype=np.int64))
    li = (np.asarray(inputs['hand_loop_verts_inds_left'], dtype=np.int64),
          np.asarray(inputs['hand_loop_verts_inds_right'], dtype=np.int64))
    fa = (np.asarray(inputs['hand_faces_left'], dtype=np.int64),
          np.asarray(inputs['hand_faces_right'], dtype=np.int64))

    ptsK = np.full((NBD, 3, PPAD), PADV, np.float32)
    for d in range(2):
        h = verts[:, hi[d]]                                     # [B,250,3]
        lid = verts[:, li[d]].mean(axis=1, keepdims=True)       # [B,1,3]
        p = np.concatenate([h, lid], axis=1)                    # [B,251,3]
        sl = slice(d * B, d * B + B)
        ptsK[sl, :, :P] = p.transpose(0, 2, 1)

    blob = _cache.setdefault("blob", np.empty(L_BLOB, np.float32))
    blob[:L_PTS] = ptsK.reshape(-1)
    fb = np.full((2, 3, FP), -1.0, np.float32)
    for h_ in range(2):
        fb[h_, :, :F] = fa[h_].T
    blob[OFF_FACES:OFF_QI] = fb.reshape(-1)
    if not _cache.get("blob_const"):
        qiota = np.stack([np.arange(128, dtype=np.float32),
                          np.arange(128, 256, dtype=np.float32)], axis=1)
        blob[OFF_QI:OFF_IR] = qiota.reshape(-1)
        blob[OFF_IR:] = np.arange(128, dtype=np.float32)
        _cache["blob_const"] = True
    return {"blob": blob}


# --------------------------------------------------------------------------
# device kernel
# --------------------------------------------------------------------------

def _kernel_body(tc, blob_d, loss_d):
    import concourse.mybir as mybir
    nc = tc.nc
    fp32 = mybir.dt.float32
    AF = mybir.ActivationFunctionType
    OP = mybir.AluOpType
    AX = mybir.AxisListType.X
    TT = nc.vector.tensor_tensor
    TS = nc.vector.tensor_scalar
    STT = nc.vector.scalar_tensor_tensor

    with (
        tc.tile_pool(name="const", bufs=1) as cpool,
        tc.tile_pool(name="dram", bufs=1, space="DRAM") as dpool,
    ):
        frhs_dram = dpool.tile([NBD, 5, 7, FP], fp32)

        ptsK = cpool.tile([128, 4, PPAD], fp32)
        S = cpool.tile([128, 5, PPAD], fp32)     # lhsT rows per bd (bd-major)
        M = cpool.tile([128, 5, PPAD], fp32)     # min-dist cols per bd
        qiota = cpool.tile([128, 2], fp32)
        ones = cpool.tile([128, 1], fp32)
        sacc = cpool.tile([128, NBLK], fp32)
        minda = cpool.tile([128, NBLK], fp32)

        nc.sync.dma_start(ptsK[:, 0:3, :], blob_d[0:L_PTS])
        nc.sync.dma_start(qiota[:], blob_d[OFF_QI:OFF_QI + 256])
        nc.vector.memset(ones[:], 1.0)

        # derive |p|^2 plane on device
        sq = cpool.tile([128, 3, PPAD], fp32)
        s01 = cpool.tile([128, PPAD], fp32)
        nc.scalar.activation(sq[:], ptsK[:, 0:3, :], AF.Square)
        TT(s01[:], sq[:, 0, :], sq[:, 1, :], OP.add)
        TT(ptsK[:, 3, :], s01[:], sq[:, 2, :], OP.add)

        TS(S[:, 0:3, :], ptsK[:, 0:3, :], -2.0, None, OP.mult)
        nc.vector.memset(S[:, 3, :], 1.0)
        nc.scalar.copy(S[:, 4, :], ptsK[:, 3, :])
        nc.scalar.copy(M[:, 0:4, :], ptsK[:, 0:4, :])
        nc.vector.memset(M[:, 4, :], 1.0)

        # ---------------- prep: per-face constants -> frhs_dram ------------
        with (
            tc.tile_pool(name="prep", bufs=1) as pr,
            tc.tile_pool(name="prep2", bufs=2) as pr2,
            tc.tile_pool(name="prep_ps", bufs=1, space="PSUM") as pp,
        ):
            faces0 = pr.tile([1, 3 * FP], fp32)
            faces1 = pr.tile([1, 3 * FP], fp32)
            irow = pr.tile([1, 128], fp32)
            onesr = pr.tile([1, 128], fp32)
            nc.sync.dma_start(faces0[:], blob_d[OFF_FACES:OFF_FACES + L_FACES])
            nc.sync.dma_start(
                faces1[:], blob_d[OFF_FACES + L_FACES:OFF_FACES + 2 * L_FACES])
            nc.sync.dma_start(irow[:], blob_d[OFF_IR:OFF_IR + 128])
            nc.vector.memset(onesr[:], 1.0)

            # identity (for PE transpose) from iota compare
            psumI = pp.tile([128, 128], fp32, tag="pI")
            nc.tensor.matmul(psumI[:], onesr[:], irow[:])
            ident = pr.tile([128, 128], fp32)
            TS(ident[:], psumI[:], qiota[:, 0:1], None, OP.is_equal)

            # gvo[qhat, t, bd]: vertex values transposed; t = 2*coord + half
            gvo = pr.tile([128, 6, 128], fp32)
            for t in range(6):
                pt = pp.tile([128, 128], fp32, tag="pT")
                nc.tensor.transpose(
                    pt[:], ptsK[:, t // 2, (t % 2) * 128:(t % 2) * 128 + 128],
                    ident[:])
                nc.scalar.copy(gvo[:, t, :], pt[:])

            ALG = pr.tile([128, 5, 7, FP], fp32)

            for h in range(2):          # h = hand whose faces we gather
                d = 1 - h               # ALG dir-block receiving them
                rows = slice(d * B, d * B + B)
                cols = slice(h * B, h * B + B)
                fsb = faces0 if h == 0 else faces1
                psumF = pp.tile([128, 3, FP], fp32, tag="pF")
                for nsp in range(3):
                    nc.tensor.matmul(psumF[:, nsp, :], onesr[:],
                                     fsb[:, nsp * FP:(nsp + 1) * FP])
                G0 = pr.tile([128, 3, FP], fp32, tag="G0")
                G1 = pr.tile([128, 3, FP], fp32, tag="G1")
                TS(G0[:], psumF[:], qiota[:, 0:1], None, OP.is_equal)
                TS(G1[:], psumF[:], qiota[:, 1:2], None, OP.is_equal)
                G = (G0, G1)
                for k in range(3):
                    psumT = pp.tile([B, 3, FP], fp32, tag="pTRI")
                    for nsp in range(3):
                        for c2 in range(2):
                            nc.tensor.matmul(psumT[:, nsp, :],
                                             gvo[:, 2 * k + c2, cols],
                                             G[c2][:, nsp, :],
                                             start=(c2 == 0), stop=(c2 == 1))
                    for g in range(3):
                        nc.scalar.copy(ALG[rows, k, g, :], psumT[:, g, :])

            def Ap(k): return ALG[:, k, 0, :]
            def Bp(k): return ALG[:, k, 1, :]
            def Cp(k): return ALG[:, k, 2, :]

            # midpoint groups (A+B)/2, (B+C)/2, (C+A)/2
            for g, (X, Y) in ((3, (Ap, Bp)), (4, (Bp, Cp)), (5, (Cp, Ap))):
                for k in range(3):
                    t1 = pr2.tile([128, FP], fp32, tag="mid")
                    TT(t1[:], X(k), Y(k), OP.add)
                    TS(ALG[:, k, g, :], t1[:], 0.5, None, OP.mult)

            # row 3: |A|^2,|B|^2,|C|^2, A.B, B.C, C.A
            for g, (X, Y) in enumerate(((Ap, Ap), (Bp, Bp), (Cp, Cp),
                                        (Ap, Bp), (Bp, Cp), (Cp, Ap))):
                t1 = pr2.tile([128, FP], fp32, tag="d1")
                t2 = pr2.tile([128, FP], fp32, tag="d2")
                t3 = pr2.tile([128, FP], fp32, tag="d3")
                t4 = pr2.tile([128, FP], fp32, tag="d4")
                TT(t1[:], X(0), Y(0), OP.mult)
                TT(t2[:], X(1), Y(1), OP.mult)
                TT(t3[:], t1[:], t2[:], OP.add)
                TT(t4[:], X(2), Y(2), OP.mult)
                TT(ALG[:, 3, g, :], t3[:], t4[:], OP.add)

            # group 6: n = (B-A)x(C-A) = AxB+BxC+CxA, d0 = A.n
            E1 = pr.tile([128, 3, FP], fp32)
            E2 = pr.tile([128, 3, FP], fp32)
            N3 = pr.tile([128, 3, FP], fp32)
            for k in range(3):
                TT(E1[:, k, :], Bp(k), Ap(k), OP.subtract)
                TT(E2[:, k, :], Cp(k), Ap(k), OP.subtract)
            for k, (i1, i2) in ((0, (1, 2)), (1, (2, 0)), (2, (0, 1))):
                c1 = pr2.tile([128, FP], fp32, tag="cx1")
                c2_ = pr2.tile([128, FP], fp32, tag="cx2")
                TT(c1[:], E1[:, i1, :], E2[:, i2, :], OP.mult)
                TT(c2_[:], E1[:, i2, :], E2[:, i1, :], OP.mult)
                TT(N3[:, k, :], c1[:], c2_[:], OP.subtract)
            d1 = pr2.tile([128, FP], fp32, tag="d1")
            d2 = pr2.tile([128, FP], fp32, tag="d2")
            d3 = pr2.tile([128, FP], fp32, tag="d3")
            d4 = pr2.tile([128, FP], fp32, tag="d4")
            TT(d1[:], Ap(0), N3[:, 0, :], OP.mult)
            TT(d2[:], Ap(1), N3[:, 1, :], OP.mult)
            TT(d3[:], d1[:], d2[:], OP.add)
            TT(d4[:], Ap(2), N3[:, 2, :], OP.mult)
            TT(ALG[:, 3, 6, :], d3[:], d4[:], OP.add)
            for k in range(3):
                TS(ALG[:, k, 6, :], N3[:, k, :], 0.5, None, OP.mult)

            nc.vector.memset(ALG[:, 4, 0:6, :], 1.0)
            nc.vector.memset(ALG[:, 4, 6, :], 0.0)

            nc.sync.dma_start(frhs_dram[:], ALG[:])

        # ---------------- main loop (proven 8-core math, 256 blocks) -------
        with (
            tc.tile_pool(name="store", bufs=1) as spool,
            tc.tile_pool(name="stage", bufs=2) as stpool,
            tc.tile_pool(name="iface", bufs=2) as ipool,
            tc.tile_pool(name="dve", bufs=1) as vpool,
        ):
            denoms = spool.tile([128, SUPER, FP], fp32)
            tts = spool.tile([128, SUPER, FP], fp32)
            bx = spool.tile([128, SUPER, FP], fp32)
            by = spool.tile([128, SUPER, FP], fp32)
            bz = spool.tile([128, SUPER, FP], fp32)

            def pass_a(ppool, lstage, bd0, i, j):
                bd, ch = divmod(i, 2)
                if ch == 0:
                    fstage = stpool.tile([5, 7, FP], fp32, tag="fstage")
                    mstage = stpool.tile([5, PPAD], fp32, tag="mstage")
                    nc.sync.dma_start(fstage[:], frhs_dram[bd])
                    other = (bd + B) % NBD
                    nc.sync.dma_start(mstage[:], M[other:other + 1, :, :])
                    pass_a.stage = (fstage, mstage)
                fstage, mstage = pass_a.stage
                lhs = lstage[:, bd - bd0, ch * 128:(ch + 1) * 128]

                wind = ppool.tile([128, 7, FP], fp32, tag="wind")
                md = ppool.tile([128, PPAD], fp32, tag="md")
                for g in range(7):
                    nc.tensor.matmul(wind[:, g, :], lhs, fstage[:, g, :])
                nc.tensor.matmul(md[:, :P], lhs, mstage[:, :P])

                mind = vpool.tile([128, 1], fp32, tag="mind")
                nc.vector.tensor_reduce(mind[:], md[:, :P], AX, OP.min)
                TS(minda[:, i:i + 1], mind[:], 0.0, None, OP.max)

                rl = ipool.tile([128, 3, FP], fp32, tag="rl")
                rl2 = ipool.tile([128, 3, FP], fp32, tag="rl2")
                nc.scalar.activation(rl[:], wind[:, 0:3, :], AF.Relu)
                nc.scalar.activation(rl2[:], rl[:], AF.Sqrt)
                dets = ipool.tile([128, FP], fp32, tag="dets")
                nc.scalar.copy(dets[:], wind[:, 6, :])

                r4 = vpool.tile([128, FP], fp32, tag="r4")
                s5 = vpool.tile([128, FP], fp32, tag="s5")
                u = vpool.tile([128, FP], fp32, tag="u")
                v = vpool.tile([128, FP], fp32, tag="v")
                w = vpool.tile([128, FP], fp32, tag="w")
                t6 = vpool.tile([128, FP], fp32, tag="t6")
                TT(r4[:], wind[:, 4, :], rl2[:, 0, :], OP.mult)
                TT(s5[:], wind[:, 5, :], rl2[:, 1, :], OP.mult)
                TT(u[:], rl2[:, 0, :], rl2[:, 1, :], OP.mult)
                TT(v[:], u[:], wind[:, 3, :], OP.add)
                TT(w[:], v[:], rl2[:, 2, :], OP.mult)
                TT(t6[:], r4[:], s5[:], OP.add)
                den = denoms[:, j, :]
                TT(den, w[:], t6[:], OP.add)

                xx = ipool.tile([128, FP], fp32, tag="xx")
                yy = ipool.tile([128, FP], fp32, tag="yy")
                ss = vpool.tile([128, FP], fp32, tag="ss", bufs=2)
                rho = ipool.tile([128, FP], fp32, tag="rho")
                axd = ipool.tile([128, FP], fp32, tag="axd")
                dd = vpool.tile([128, FP], fp32, tag="dd")
                rd = vpool.tile([128, FP], fp32, tag="rd")
                nc.scalar.activation(xx[:], den, AF.Square)
                nc.scalar.activation(yy[:], dets[:], AF.Square)
                STT(ss[:], xx[:], 1e-20, yy[:], OP.add, OP.add)
                nc.scalar.activation(rho[:], ss[:], AF.Sqrt)
                nc.scalar.activation(axd[:], den, AF.Abs)
                TT(dd[:], rho[:], axd[:], OP.add)
                nc.vector.reciprocal_approx_fast(rd[:], dd[:])
                TT(tts[:, j, :], dets[:], rd[:], OP.mult)

            def pass_b(s):
                # atan2/2 = atn + [den<0]*(sign(det)*pi/2 - 2*atn), batched
                nc.scalar.activation(bx[:], tts[:], AF.Arctan)
                nc.scalar.activation(by[:], tts[:], AF.Sign)
                nc.scalar.mul(bz[:], by[:], HALF_PI)
                STT(by[:], bx[:], -2.0, bz[:], OP.mult, OP.add)
                STT(bz[:], denoms[:], 0.0, by[:], OP.is_lt, OP.mult)
                TT(by[:], bx[:], bz[:], OP.add)
                nc.vector.tensor_reduce(sacc[:, s * SUPER:(s + 1) * SUPER],
                                        by[:], AX, OP.add)

            with tc.tile_pool(name="psum", bufs=1, space="PSUM") as ppool:
                for s in range(NSG):
                    bd0 = s * (SUPER // 2)
                    lstage = stpool.tile([5, SUPER // 2, PPAD], fp32,
                                         tag="lstage")
                    for o in range(SUPER // 2):
                        nc.sync.dma_start(lstage[:, o, :],
                                          S[bd0 + o:bd0 + o + 1, :, :])
                    for j in range(SUPER):
                        pass_a(ppool, lstage, bd0, s * SUPER + j, j)
                    tc.no_sync_barrier()
                    pass_b(s)
                    tc.no_sync_barrier()

            # ---------------- final: depth * inside, partition-reduce ------
            inside = cpool.tile([128, NBLK], fp32)
            depth = cpool.tile([128, NBLK], fp32)
            contrib = cpool.tile([128, NBLK], fp32)
            beps = cpool.tile([128, 1], fp32)
            nc.vector.memset(beps[:], 1e-12)
            TS(inside[:], sacc[:], HALF_PI, None, OP.is_gt)
            nc.scalar.activation(depth[:], minda[:], AF.Sqrt, bias=beps[:])
            TT(contrib[:], depth[:], inside[:], OP.mult)

            with tc.tile_pool(name="psum2", bufs=1, space="PSUM") as p2:
                lp0 = p2.tile([128, 1], fp32, tag="lp0")
                lp1 = p2.tile([128, 1], fp32, tag="lp1")
                nc.tensor.matmul(lp0[:], contrib[:, 0:128], ones[:])
                nc.tensor.matmul(lp1[:], contrib[:, 128:256], ones[:])
                loss_sb = cpool.tile([128, 2], fp32)
                nc.scalar.copy(loss_sb[:, 0:1], lp0[:])
                nc.scalar.copy(loss_sb[:, 1:2], lp1[:])
                nc.sync.dma_start(loss_d[:], loss_sb[:])


def _build():
    if "nc" in _cache:
        return _cache["nc"]
    import concourse.bacc as bacc
    import concourse.mybir as mybir
    import concourse.tile as tile

    nc = bacc.Bacc("TRN2", target_bir_lowering=False, debug=False,
                   num_devices=1)
    fp32 = mybir.dt.float32
    blob_d = nc.dram_tensor("blob", (L_BLOB,), fp32,
                            kind="ExternalInput").ap()
    loss_d = nc.dram_tensor("loss", (128, 2), fp32,
                            kind="ExternalOutput").ap()

    with tile.TileContext(nc) as tc:
        _kernel_body(tc, blob_d, loss_d)
    nc.compile()
    _cache["nc"] = nc
    return nc


# --------------------------------------------------------------------------
# cached single-device dispatch (PJRT via axon)
# --------------------------------------------------------------------------

def _get_runner(nc):
    if "run" in _cache:
        return _cache["run"]
    import jax
    from concourse import bass2jax, mybir
    from concourse.bass2jax import _bass_exec_p, partition_id_tensor

    bass2jax.install_neuronx_cc_hook()
    partition_name = (nc.partition_id_tensor.name
                      if nc.partition_id_tensor else None)
    in_names, in_shapes, out_names, out_avals = [], [], [], []
    for alloc in nc.m.functions[0].allocations:
        if not isinstance(alloc, mybir.MemoryLocationSet):
            continue
        name = alloc.memorylocations[0].name
        if alloc.kind == "ExternalInput":
            if name != partition_name:
                in_names.append(name)
                in_shapes.append((tuple(alloc.tensor_shape),
                                  mybir.dt.np(alloc.dtype)))
        elif alloc.kind == "ExternalOutput":
            out_names.append(name)
            out_avals.append(jax.core.ShapedArray(
                tuple(alloc.tensor_shape), mybir.dt.np(alloc.dtype)))
    n_params = len(in_names)
    in_names_full = list(in_names) + list(out_names)
    if partition_name is not None:
        in_names_full.append(partition_name)
    donate = tuple(range(n_params, n_params + len(out_names)))
    dbg_zero = {}
    if nc.dbg_addr is not None:
        assert not nc.dbg_callbacks
        dbg_zero = {nc.dbg_addr.name: np.zeros((1, 2), np.uint32)}

    def _body(*args):
        operands = list(args)
        if partition_name is not None:
            operands.append(partition_id_tensor())
        return tuple(_bass_exec_p.bind(
            *operands, out_avals=tuple(out_avals),
            in_names=tuple(in_names_full), out_names=tuple(out_names),
            lowering_input_output_aliases=(),
            sim_require_finite=True, sim_require_nnan=True, nc=nc))

    fn = jax.jit(_body, donate_argnums=donate, keep_unused=True)
    out_shapes = [(tuple(a.shape), a.dtype) for a in out_avals]

    def run(in_map):
        full = {**in_map, **dbg_zero}
        args = [np.ascontiguousarray(full[n]) for n in in_names]
        zeros = [np.zeros(sh, dt) for sh, dt in out_shapes]
        outs = fn(*args, *zeros)
        # Pipeline the D2H copy behind the execute instead of paying a
        # separate blocking round trip for it (saves ~30ms on this link).
        for o in outs:
            try:
                o.copy_to_host_async()
            except Exception:
                pass
        return {name: np.asarray(outs[k]) for k, name in enumerate(out_names)}

    _cache["run"] = run
    return run


# --------------------------------------------------------------------------
# entry point
# --------------------------------------------------------------------------

def kernel(**inputs) -> np.ndarray:
    global last_exec_time_ns
    in_map = _host_prep(inputs)
    nc = _build()
    run = _get_runner(nc)
    out = run(in_map)["loss"].astype(np.float32)        # [128, 2]
    flat = np.concatenate([out[:, 0], out[:, 1]])       # block i = bd*2+ch
    per_bd = flat.reshape(NBD, 2).sum(axis=1)
    loss = per_bd[:B] + per_bd[B:]
    last_exec_time_ns = None
    return loss.astype(np.float32)
